# revision 11
# baseline (speedup 1.0000x reference)
"""Bipartite GNN attention kernel for Trainium2, SPMD across 8 NeuronCores.

Math (per reference):
  u = user @ W_u.T + b_u ; v = item @ W_v.T + b_v
  learn_user = softmax((u @ v.T) * UV_adj * scale, axis=1) @ v + u
  learn_item = softmax((v @ u.T) * VU_adj * scale, axis=1) @ u + v

Sharding: core i owns rows [i*1024, (i+1)*1024) of BOTH outputs.
Each core computes the full projected feature matrix it contracts against
(replicated), its own row block of scores in transposed layout
S^T[b, r] (b = column index on partitions), applies mask*exp, reduces the
softmax denominator with a ones-matmul, and accumulates P^T.T @ v into
PSUM over all 64 column chunks.

All matmuls run as float32r (FP22 truncation, 1 cyc/row at free>=256).
Softmax skips max-subtraction (scores bounded ~|6|, exp is safe in f32),
which matches jax.nn.softmax exactly in exact arithmetic.

Host-side prep: feeds pre-transposed userT/itemT/W_uT/W_vT so the device
never transposes inputs; the mask for the UV direction is read from
VU_adj columns (= UV_adj transposed) and vice versa.
"""

import sys

sys.path.insert(0, "/opt/trn_rl_repo")

import numpy as np

import concourse.bacc as bacc
import concourse.bass as bass
import concourse.mybir as mybir
import concourse.tile as tile
from concourse.bass_utils import run_bass_kernel_spmd

N = 8192          # users == items
H = 512           # hidden
NCORES = 8
RB = N // NCORES  # 1024 rows per core per direction
KH = H // 128     # 4 h-chunks
NB = N // 128     # 64 column chunks
NRB = RB // 512   # 2 r-blocks of 512
SCALE = float(1.0 / np.sqrt(np.float32(H)))

F32 = mybir.dt.float32
F32R = mybir.dt.float32r


def _r(ap):
    return ap.bitcast(F32R)


def build_nc():
    nc = bacc.Bacc("TRN2", target_bir_lowering=False, debug=False)

    userT = nc.declare_dram_parameter("userT", [H, N], F32, isOutput=False)
    itemT = nc.declare_dram_parameter("itemT", [H, N], F32, isOutput=False)
    userT_blk = nc.declare_dram_parameter("userT_blk", [H, RB], F32, isOutput=False)
    itemT_blk = nc.declare_dram_parameter("itemT_blk", [H, RB], F32, isOutput=False)
    maskA = nc.declare_dram_parameter("maskA", [N, RB], F32, isOutput=False)
    maskB = nc.declare_dram_parameter("maskB", [N, RB], F32, isOutput=False)
    W_uT = nc.declare_dram_parameter("W_uT", [H, H], F32, isOutput=False)
    W_vT = nc.declare_dram_parameter("W_vT", [H, H], F32, isOutput=False)
    b_u_p = nc.declare_dram_parameter("b_u_p", [128, KH], F32, isOutput=False)
    b_v_p = nc.declare_dram_parameter("b_v_p", [128, KH], F32, isOutput=False)
    ident = nc.declare_dram_parameter("ident", [128, 128], F32, isOutput=False)
    ones_p = nc.declare_dram_parameter("ones_p", [128, 1], F32, isOutput=False)
    out = nc.declare_dram_parameter("out", [2 * RB, H], F32, isOutput=True)

    with tile.TileContext(nc) as tc:
        with (
            tc.tile_pool(name="big", bufs=1) as big,          # vT_full / uT_full
            tc.tile_pool(name="blk", bufs=1) as blk,          # uT_blk etc
            tc.tile_pool(name="wts", bufs=1) as wts,
            tc.tile_pool(name="stream", bufs=6) as stream,    # proj rhs tiles
            tc.tile_pool(name="mask", bufs=2) as maskp,
            tc.tile_pool(name="pbuf", bufs=2) as pbuf,
            tc.tile_pool(name="vchunk", bufs=2) as vchp,
            tc.tile_pool(name="outs", bufs=2) as outsp,
            tc.tile_pool(name="small", bufs=1) as small,
            tc.tile_pool(name="ps_s", bufs=2, space="PSUM") as ps_s,    # 2 banks
            tc.tile_pool(name="ps_tr", bufs=1, space="PSUM") as ps_tr,  # 1 bank
            tc.tile_pool(name="ps_agg", bufs=1, space="PSUM") as ps_agg,  # 4 banks
            tc.tile_pool(name="ps_rs", bufs=1, space="PSUM") as ps_rs,  # 1 bank
        ):
            identity = small.tile([128, 128], F32R, tag="ident")
            nc.sync.dma_start(identity[:], ident[:].bitcast(F32R))
            one11 = small.tile([1, 1], F32, tag="one11")
            nc.vector.memset(one11[:], 1.0)
            ones = small.tile([128, 1], F32R, tag="ones")
            nc.sync.dma_start(ones[:], ones_p[:].bitcast(F32R))
            zbias = small.tile([128, 1], F32, tag="zbias")
            nc.vector.memset(zbias[:], 0.0)
            bu_sb = small.tile([128, KH], F32, tag="bu")
            nc.sync.dma_start(bu_sb[:], b_u_p[:])
            bv_sb = small.tile([128, KH], F32, tag="bv")
            nc.sync.dma_start(bv_sb[:], b_v_p[:])

        # one direction: "feat" = the side being attended over (full
        # projection), "q" = the side owning output rows on this core.
            def direction(featT_dram, qT_blk_dram, w_feat_dram, w_q_dram,
                          bias_feat, bias_q, mask_dram, out_base):
                # -- load weights (transposed layout [hh, h]) --
                wf = [wts.tile([128, H], F32R, tag=f"wf{k}", name=f"wf{k}") for k in range(KH)]
                for k in range(KH):
                    nc.sync.dma_start(wf[k][:], w_feat_dram[k * 128:(k + 1) * 128, :].bitcast(F32R))
                wq = [wts.tile([128, H], F32R, tag=f"wq{k}", name=f"wq{k}") for k in range(KH)]
                for k in range(KH):
                    nc.sync.dma_start(wq[k][:], w_q_dram[k * 128:(k + 1) * 128, :].bitcast(F32R))

                # -- project full feat^T: fT[m][h=128, b] for m in 0..3 --
                fT = [big.tile([128, N], F32R, tag=f"fT{m}", name=f"fT{m}") for m in range(KH)]
                for n in range(N // 512):
                    ft_in = [stream.tile([128, 512], F32R, tag="ft_in", name=f"ft{n}_{k}")
                             for k in range(KH)]
                    for k in range(KH):
                        nc.sync.dma_start(
                            ft_in[k][:],
                            featT_dram[k * 128:(k + 1) * 128,
                                       n * 512:(n + 1) * 512].bitcast(F32R))
                    for m in range(KH):
                        ps = ps_s.tile([128, 512], F32, tag="s")
                        for k in range(KH):
                            nc.tensor.matmul(
                                ps[:], _r(wf[k][:, m * 128:(m + 1) * 128]),
                                _r(ft_in[k][:]),
                                start=(k == 0), stop=(k == KH - 1))
                        nc.vector.tensor_scalar(
                            out=fT[m][:, n * 512:(n + 1) * 512], in0=ps[:],
                            scalar1=bias_feat[:, m:m + 1], scalar2=None,
                            op0=mybir.AluOpType.add)

                # -- project q^T block: qT[m][h=128, r=RB] --
                qT = [blk.tile([128, RB], F32R, tag=f"qT{m}", name=f"qT{m}") for m in range(KH)]
                for n in range(RB // 512):
                    qt_in = [stream.tile([128, 512], F32R, tag="ft_in", name=f"qt{n}_{k}")
                             for k in range(KH)]
                    for k in range(KH):
                        nc.sync.dma_start(
                            qt_in[k][:],
                            qT_blk_dram[k * 128:(k + 1) * 128,
                                        n * 512:(n + 1) * 512].bitcast(F32R))
                    for m in range(KH):
                        ps = ps_s.tile([128, 512], F32, tag="s")
                        for k in range(KH):
                            nc.tensor.matmul(
                                ps[:], _r(wq[k][:, m * 128:(m + 1) * 128]),
                                _r(qt_in[k][:]),
                                start=(k == 0), stop=(k == KH - 1))
                        nc.vector.tensor_scalar(
                            out=qT[m][:, n * 512:(n + 1) * 512], in0=ps[:],
                            scalar1=bias_q[:, m:m + 1], scalar2=None,
                            op0=mybir.AluOpType.add)

                # -- q block row-major [r=128, H] x8 via PE transpose --
                q_row = [blk.tile([128, H], F32R, tag=f"qrow{rs}", name=f"qrow{rs}")
                         for rs in range(RB // 128)]
                for rs in range(RB // 128):
                    tp = ps_tr.tile([128, 512], F32R, tag="tr")
                    for m in range(KH):
                        nc.tensor.transpose(
                            tp[:, m * 128:(m + 1) * 128],
                            qT[m][:, rs * 128:(rs + 1) * 128],
                            identity[:])
                    nc.vector.tensor_copy(q_row[rs][:], tp[:])

                # -- main attention loop --
                for rb in range(NRB):
                    agg = ps_agg.tile([128, KH, 512], F32, tag="agg")
                    rsum = ps_rs.tile([1, 512], F32, tag="rs")
                    for b in range(NB):
                        # feat chunk row-major [b=128, H] via PE transpose
                        tp = ps_tr.tile([128, 512], F32R, tag="tr")
                        for m in range(KH):
                            nc.tensor.transpose(
                                tp[:, m * 128:(m + 1) * 128],
                                fT[m][:, b * 128:(b + 1) * 128],
                                identity[:])
                        v_chunk = vchp.tile([128, 512], F32R, tag="vch")
                        nc.vector.tensor_copy(v_chunk[:], tp[:])

                        # scores S^T[b=128, r=512]
                        sps = ps_s.tile([128, 512], F32, tag="s")
                        for m in range(KH):
                            nc.tensor.matmul(
                                sps[:], _r(fT[m][:, b * 128:(b + 1) * 128]),
                                _r(qT[m][:, rb * 512:(rb + 1) * 512]),
                                start=(m == 0), stop=(m == KH - 1))

                        # mask multiply then exp(scale * x)
                        mt = maskp.tile([128, 512], F32, tag="mk")
                        nc.sync.dma_start(
                            mt[:], mask_dram[b * 128:(b + 1) * 128,
                                             rb * 512:(rb + 1) * 512])
                        p_sb = pbuf.tile([128, 512], F32R, tag="p")
                        nc.vector.tensor_tensor(
                            out=p_sb[:], in0=sps[:], in1=mt[:],
                            op=mybir.AluOpType.mult)
                        nc.scalar.activation(
                            p_sb[:], p_sb[:], mybir.ActivationFunctionType.Exp,
                            bias=zbias[:], scale=SCALE)

                        # denominator: rsum[1, r] += ones.T @ P
                        nc.tensor.matmul(
                            rsum[:], _r(ones[:]), _r(p_sb[:]),
                            start=(b == 0), stop=(b == NB - 1))
                        # aggregation: agg[rs] += P[:, rs].T @ v_chunk
                        for rs in range(4):
                            nc.tensor.matmul(
                                agg[:, rs, :],
                                _r(p_sb[:, rs * 128:(rs + 1) * 128]),
                                _r(v_chunk[:]),
                                start=(b == 0), stop=(b == NB - 1))

                    # epilogue: out rows = agg / rsum + q_row
                    rsum_sb = small.tile([1, 512], F32, tag="rsum_sb")
                    nc.vector.reciprocal(rsum_sb[:], rsum[:])
                    tp2 = ps_tr.tile([128, 512], F32, tag="tr")
                    for rs in range(4):
                        nc.tensor.transpose(
                            tp2[:, rs:rs + 1],
                            rsum_sb[:, rs * 128:(rs + 1) * 128],
                            one11[:])
                    recip = small.tile([128, 4], F32, tag="recip")
                    nc.vector.tensor_copy(recip[:], tp2[:, 0:4])
                    for rs in range(4):
                        o_sb = outsp.tile([128, H], F32, tag="o")
                        nc.vector.scalar_tensor_tensor(
                            out=o_sb[:], in0=agg[:, rs, :],
                            scalar=recip[:, rs:rs + 1],
                            in1=q_row[rb * 4 + rs][:],
                            op0=mybir.AluOpType.mult,
                            op1=mybir.AluOpType.add)
                        row0 = out_base + rb * 512 + rs * 128
                        nc.sync.dma_start(out[row0:row0 + 128, :], o_sb[:])

            # UV direction: q = user rows, feat = item, mask^T = VU_adj cols
            direction(itemT, userT_blk, W_vT, W_uT, bv_sb, bu_sb, maskA, 0)
            # VU direction: q = item rows, feat = user, mask^T = UV_adj cols
            direction(userT, itemT_blk, W_uT, W_vT, bu_sb, bv_sb, maskB, RB)

    nc.compile()
    return nc


_NC_CACHE = None
TRACE = False
LAST_RESULT = None


def kernel(user, item, UV_adj, VU_adj, W_u, b_u, W_v, b_v):
    global _NC_CACHE
    user = np.asarray(user, dtype=np.float32)
    item = np.asarray(item, dtype=np.float32)
    UV_adj = np.asarray(UV_adj, dtype=np.float32)
    VU_adj = np.asarray(VU_adj, dtype=np.float32)
    W_u = np.asarray(W_u, dtype=np.float32)
    W_v = np.asarray(W_v, dtype=np.float32)
    b_u = np.asarray(b_u, dtype=np.float32)
    b_v = np.asarray(b_v, dtype=np.float32)

    userT = np.ascontiguousarray(user.T)
    itemT = np.ascontiguousarray(item.T)
    W_uT = np.ascontiguousarray(W_u.T)
    W_vT = np.ascontiguousarray(W_v.T)
    b_u_p = np.ascontiguousarray(b_u.reshape(KH, 128).T)
    b_v_p = np.ascontiguousarray(b_v.reshape(KH, 128).T)
    ident = np.eye(128, dtype=np.float32)

    in_maps = []
    for i in range(NCORES):
        sl = slice(i * RB, (i + 1) * RB)
        in_maps.append({
            "userT": userT,
            "itemT": itemT,
            "userT_blk": np.ascontiguousarray(userT[:, sl]),
            "itemT_blk": np.ascontiguousarray(itemT[:, sl]),
            "maskA": np.ascontiguousarray(VU_adj[:, sl]),
            "maskB": np.ascontiguousarray(UV_adj[:, sl]),
            "W_uT": W_uT,
            "W_vT": W_vT,
            "b_u_p": b_u_p,
            "b_v_p": b_v_p,
            "ident": ident,
            "ones_p": np.ones((128, 1), dtype=np.float32),
        })

    global LAST_RESULT
    if _NC_CACHE is None:
        _NC_CACHE = build_nc()
    res = run_bass_kernel_spmd(_NC_CACHE, in_maps, core_ids=list(range(NCORES)),
                               trace=TRACE)
    LAST_RESULT = res
    results = res.results
    learn_user = np.concatenate([results[i]["out"][:RB] for i in range(NCORES)], 0)
    learn_item = np.concatenate([results[i]["out"][RB:] for i in range(NCORES)], 0)
    return (learn_user, learn_item)


if __name__ == "__main__":
    nc = build_nc()
    print("built ok")
